# revision 7
# baseline (speedup 1.0000x reference)
"""AVSL-Graph fused kernel for Trainium2 (8 NeuronCores, data-parallel over batch).

Computation (per reference):
  for each level l in {0,1,2}:
    cam_l[b,r,hw] = sum_c w_l[r,c] * fmap_l[b,c,hw]          (1x1-conv GEMM)
    emb_l[b,r]    = mean_hw(cam_l) + bias_l[r]               (== pooled @ w.T + b)
    cert_l[b,r]   = std_hw(cam_l, ddof=1)
  link_l = sum_b  norm(pool2x2(cam_l))[b] @ norm(cam_{l+1})[b].T / B

Sharding: batch B=64 split 8 ways (8 samples/core). Each core computes its
embedding/certainty rows and an unnormalized link partial (sum over its local
samples, accumulated in PSUM); the host sums the 8 partials and divides by B.
"""

import os
import sys

import numpy as np

for _p in ("/opt/trn_rl_repo", "/root/.axon_site/_ro/trn_rl_repo"):
    if os.path.isdir(_p) and _p not in sys.path:
        sys.path.append(_p)

B = 64
R = 128
NCORES = 8
BLOC = B // NCORES  # 8 samples per core
CH = (512, 1024, 2048)
HW = (784, 196, 49)  # 28*28, 14*14, 7*7
KC = (4, 8, 16)  # contraction chunks of 128 per level
EPS = 1e-12

_CACHE = {}


def _build():
    import concourse.bacc as bacc
    import concourse.tile as tile
    from concourse import masks, mybir

    f32 = mybir.dt.float32
    AX = mybir.AxisListType
    AF = mybir.ActivationFunctionType

    nc = bacc.Bacc("TRN2", target_bir_lowering=False, debug=False)

    fm = [
        nc.dram_tensor(f"fmap{l}", (BLOC, CH[l], HW[l]), f32, kind="ExternalInput")
        for l in range(3)
    ]
    wT = [
        nc.dram_tensor(f"w{l}T", (CH[l], R), f32, kind="ExternalInput")
        for l in range(3)
    ]
    bias = nc.dram_tensor("bias", (R, 3), f32, kind="ExternalInput")
    emb_d = [
        nc.dram_tensor(f"emb{l}", (R, BLOC), f32, kind="ExternalOutput")
        for l in range(3)
    ]
    cert_d = [
        nc.dram_tensor(f"cert{l}", (R, BLOC), f32, kind="ExternalOutput")
        for l in range(3)
    ]
    link_d = [
        nc.dram_tensor(f"link{l}", (R, R), f32, kind="ExternalOutput")
        for l in range(2)
    ]

    with tile.TileContext(nc) as tc:
        with (
            tc.tile_pool(name="const", bufs=1) as const,
            tc.tile_pool(name="f0p", bufs=3) as f0p,
            tc.tile_pool(name="f1p", bufs=2) as f1p,
            tc.tile_pool(name="f2p", bufs=1) as f2p,
            tc.tile_pool(name="work", bufs=2) as work,
            tc.tile_pool(name="stats", bufs=2) as stats,
            tc.tile_pool(name="outp", bufs=1) as outp,
            tc.tile_pool(name="ps_cam0", bufs=1, space="PSUM") as ps_cam0,
            tc.tile_pool(name="ps_cam1", bufs=1, space="PSUM") as ps_cam1,
            tc.tile_pool(name="ps_cam2", bufs=1, space="PSUM") as ps_cam2,
            tc.tile_pool(name="ps_link", bufs=1, space="PSUM") as ps_link,
            tc.tile_pool(name="ps_tr", bufs=2, space="PSUM") as ps_tr,
        ):
            # ---- constants ----
            wt = []
            for l in range(3):
                t = const.tile([128, KC[l], 128], f32, tag=f"wt{l}", name=f"wt{l}")
                nc.sync.dma_start(t[:], wT[l][:].rearrange("(k p) r -> p k r", p=128))
                wt.append(t)
            bias_t = const.tile([128, 3], f32, tag="bias")
            nc.sync.dma_start(bias_t[:], bias[:])
            ident = const.tile([128, 128], f32, tag="ident")
            masks.make_identity(nc, ident[:])

            # ---- output accumulators in SBUF ----
            emb_t = [outp.tile([128, BLOC], f32, tag=f"emb{l}", name=f"emb_t{l}") for l in range(3)]
            cert_t = [outp.tile([128, BLOC], f32, tag=f"cert{l}", name=f"cert_t{l}") for l in range(3)]

            # ---- level 2 GEMM, all 8 samples at once (free dim = (b, hw) = 392) ----
            f2t = f2p.tile([128, KC[2], BLOC, HW[2]], f32, tag="f2")
            fm2v = fm[2][:].rearrange("b (k p) hw -> p k b hw", p=128)
            for k in range(KC[2]):
                nc.sync.dma_start(f2t[:, k, :, :], fm2v[:, k, :, :])
            cam2 = ps_cam2.tile([128, BLOC, HW[2]], f32, tag="cam2")
            for k in range(KC[2]):
                nc.tensor.matmul(
                    cam2[:],
                    wt[2][:, k, :],
                    f2t[:, k, :, :],
                    start=(k == 0),
                    stop=(k == KC[2] - 1),
                )

            link0_t = ps_link.tile([128, 128], f32, tag="link0")
            link1_t = ps_link.tile([128, 128], f32, tag="link1")

            cam1 = None
            for b in range(BLOC):
                # ---- level 1 GEMM per pair of samples (free dim 392) ----
                if b % 2 == 0:
                    f1t = f1p.tile([128, KC[1], 2, HW[1]], f32, tag="f1")
                    for j in range(2):
                        nc.sync.dma_start(
                            f1t[:, :, j, :],
                            fm[1][b + j].rearrange("(k p) hw -> p k hw", p=128),
                        )
                    cam1 = ps_cam1.tile([128, 2, HW[1]], f32, tag="cam1")
                    for k in range(KC[1]):
                        nc.tensor.matmul(
                            cam1[:],
                            wt[1][:, k, :],
                            f1t[:, k, :, :],
                            start=(k == 0),
                            stop=(k == KC[1] - 1),
                        )

                # ---- level 0 GEMM for this sample (2 halves of 392) ----
                f0t = f0p.tile([128, KC[0], HW[0]], f32, tag="f0")
                nc.sync.dma_start(
                    f0t[:], fm[0][b].rearrange("(k p) hw -> p k hw", p=128)
                )
                cam0 = ps_cam0.tile([128, 2, 512], f32, tag="cam0")
                for h in range(2):
                    for k in range(KC[0]):
                        nc.tensor.matmul(
                            cam0[:, h, 0:392],
                            wt[0][:, k, :],
                            f0t[:, k, h * 392 : (h + 1) * 392],
                            start=(k == 0),
                            stop=(k == KC[0] - 1),
                        )

                # ================= level 0 stats + pooled lo =================
                st0 = stats.tile([128, 2, 6], f32, tag="st0")
                for h in range(2):
                    nc.vector.bn_stats(st0[:, h, :], cam0[:, h, 0:392])
                mv0 = stats.tile([128, 2], f32, tag="mv0")
                nc.vector.bn_aggr(mv0[:], st0[:])
                nc.scalar.activation(
                    emb_t[0][:, b : b + 1], mv0[:, 0:1], AF.Identity,
                    bias=bias_t[:, 0:1], scale=1.0,
                )
                nc.scalar.activation(
                    cert_t[0][:, b : b + 1], mv0[:, 1:2], AF.Sqrt,
                    scale=float(HW[0]) / float(HW[0] - 1),
                )
                # 2x2 average pool (no 1/4 scale: it cancels in normalization)
                praw0 = work.tile([128, 196], f32, tag="praw0")
                for h in range(2):
                    nc.vector.reduce_sum(
                        praw0[:, h * 98 : (h + 1) * 98].rearrange(
                            "p (oh ow) -> p oh ow", oh=7
                        ),
                        cam0[:, h, 0:392].rearrange(
                            "p (oh dh ow dw) -> p oh ow dh dw", oh=7, dh=2, ow=14, dw=2
                        ),
                        axis=AX.XY,
                    )
                sq0 = work.tile([128, 196], f32, tag="sq0")
                q0 = stats.tile([128, 1], f32, tag="q0")
                nc.scalar.activation(sq0[:], praw0[:], AF.Square, accum_out=q0[:])
                inv_lo0 = stats.tile([128, 1], f32, tag="inv_lo0")
                nc.scalar.sqrt(inv_lo0[:], q0[:])
                nc.vector.tensor_scalar_max(inv_lo0[:], inv_lo0[:], EPS)
                nc.vector.reciprocal(inv_lo0[:], inv_lo0[:])
                lo0 = work.tile([128, 196], f32, tag="lo0")
                nc.vector.tensor_scalar_mul(lo0[:], praw0[:], inv_lo0[:])

                tr0 = ps_tr.tile([128, 4, 128], f32, tag="tr")
                nc.tensor.transpose(tr0[:, 0, :], lo0[:, 0:128], ident[:])
                nc.tensor.transpose(tr0[0:68, 1, :], lo0[:, 128:196], ident[:])

                # ================= level 1 stats + hi1 + pooled lo1 =================
                camb = cam1[:, b % 2, :]
                st1 = stats.tile([128, 6], f32, tag="st1")
                nc.vector.bn_stats(st1[:], camb)
                mv1 = stats.tile([128, 2], f32, tag="mv1")
                nc.vector.bn_aggr(mv1[:], st1[:])
                nc.scalar.activation(
                    emb_t[1][:, b : b + 1], mv1[:, 0:1], AF.Identity,
                    bias=bias_t[:, 1:2], scale=1.0,
                )
                nc.scalar.activation(
                    cert_t[1][:, b : b + 1], mv1[:, 1:2], AF.Sqrt,
                    scale=float(HW[1]) / float(HW[1] - 1),
                )
                # ||cam1||^2 = n*(var + mean^2)
                nh1 = stats.tile([128, 1], f32, tag="nh1")
                nc.vector.tensor_mul(nh1[:], mv1[:, 0:1], mv1[:, 0:1])
                nc.vector.tensor_add(nh1[:], nh1[:], mv1[:, 1:2])
                nc.scalar.activation(nh1[:], nh1[:], AF.Sqrt, scale=float(HW[1]))
                nc.vector.tensor_scalar_max(nh1[:], nh1[:], EPS)
                nc.vector.reciprocal(nh1[:], nh1[:])
                hi1 = work.tile([128, 196], f32, tag="hi1")
                nc.vector.tensor_scalar_mul(hi1[:], camb, nh1[:])
                nc.tensor.transpose(tr0[:, 2, :], hi1[:, 0:128], ident[:])
                nc.tensor.transpose(tr0[0:68, 3, :], hi1[:, 128:196], ident[:])

                praw1 = work.tile([128, 49], f32, tag="praw1")
                nc.vector.reduce_sum(
                    praw1[:].rearrange("p (oh ow) -> p oh ow", oh=7),
                    camb.rearrange(
                        "p (oh dh ow dw) -> p oh ow dh dw", oh=7, dh=2, ow=7, dw=2
                    ),
                    axis=AX.XY,
                )
                sq1 = work.tile([128, 49], f32, tag="sq1")
                q1 = stats.tile([128, 1], f32, tag="q1")
                nc.scalar.activation(sq1[:], praw1[:], AF.Square, accum_out=q1[:])
                inv_lo1 = stats.tile([128, 1], f32, tag="inv_lo1")
                nc.scalar.sqrt(inv_lo1[:], q1[:])
                nc.vector.tensor_scalar_max(inv_lo1[:], inv_lo1[:], EPS)
                nc.vector.reciprocal(inv_lo1[:], inv_lo1[:])
                lo1 = work.tile([128, 49], f32, tag="lo1")
                nc.vector.tensor_scalar_mul(lo1[:], praw1[:], inv_lo1[:])

                tr1 = ps_tr.tile([128, 4, 128], f32, tag="tr")
                nc.tensor.transpose(tr1[0:49, 0, :], lo1[:], ident[:])

                # ================= level 2 stats + hi2 =================
                cam2b = cam2[:, b, :]
                s2 = stats.tile([128, 1], f32, tag="s2")
                q2 = stats.tile([128, 1], f32, tag="q2")
                sc2 = work.tile([128, 49], f32, tag="sc2")
                nc.scalar.activation(sc2[:], cam2b, AF.Copy, accum_out=s2[:])
                sc2b = work.tile([128, 49], f32, tag="sc2b")
                nc.scalar.activation(sc2b[:], cam2b, AF.Square, accum_out=q2[:])
                nc.scalar.activation(
                    emb_t[2][:, b : b + 1], s2[:], AF.Identity,
                    bias=bias_t[:, 2:3], scale=1.0 / HW[2],
                )
                # cert2 = sqrt((q2 - s2^2/n) / (n-1))
                v2 = stats.tile([128, 1], f32, tag="v2")
                nc.vector.tensor_mul(v2[:], s2[:], s2[:])
                nc.vector.tensor_scalar_mul(v2[:], v2[:], -1.0 / HW[2])
                nc.vector.tensor_add(v2[:], v2[:], q2[:])
                nc.scalar.activation(
                    cert_t[2][:, b : b + 1], v2[:], AF.Sqrt,
                    scale=1.0 / (HW[2] - 1),
                )
                nh2 = stats.tile([128, 1], f32, tag="nh2")
                nc.scalar.sqrt(nh2[:], q2[:])
                nc.vector.tensor_scalar_max(nh2[:], nh2[:], EPS)
                nc.vector.reciprocal(nh2[:], nh2[:])
                hi2 = work.tile([128, 49], f32, tag="hi2")
                nc.vector.tensor_scalar_mul(hi2[:], cam2b, nh2[:])
                nc.tensor.transpose(tr1[0:49, 1, :], hi2[:], ident[:])

                # ---- PSUM -> SBUF copies of transposed operands ----
                loT0 = work.tile([128, 2, 128], f32, tag="loT0")
                nc.vector.tensor_copy(loT0[:, 0, :], tr0[:, 0, :])
                nc.vector.tensor_copy(loT0[0:68, 1, :], tr0[0:68, 1, :])
                hiT1 = work.tile([128, 2, 128], f32, tag="hiT1")
                nc.vector.tensor_copy(hiT1[:, 0, :], tr0[:, 2, :])
                nc.vector.tensor_copy(hiT1[0:68, 1, :], tr0[0:68, 3, :])
                loT1 = work.tile([128, 128], f32, tag="loT1")
                nc.vector.tensor_copy(loT1[0:49, :], tr1[0:49, 0, :])
                hiT2 = work.tile([128, 128], f32, tag="hiT2")
                nc.vector.tensor_copy(hiT2[0:49, :], tr1[0:49, 1, :])

                # ---- link partial accumulation over samples ----
                nc.tensor.matmul(
                    link0_t[:], loT0[:, 0, :], hiT1[:, 0, :],
                    start=(b == 0), stop=False,
                )
                nc.tensor.matmul(
                    link0_t[:], loT0[0:68, 1, :], hiT1[0:68, 1, :],
                    start=False, stop=(b == BLOC - 1),
                )
                nc.tensor.matmul(
                    link1_t[:], loT1[0:49, :], hiT2[0:49, :],
                    start=(b == 0), stop=(b == BLOC - 1),
                )

            # ---- write outputs ----
            link_sb = outp.tile([128, 2, 128], f32, tag="link_sb")
            nc.vector.tensor_copy(link_sb[:, 0, :], link0_t[:])
            nc.vector.tensor_copy(link_sb[:, 1, :], link1_t[:])
            nc.sync.dma_start(link_d[0][:], link_sb[:, 0, :])
            nc.sync.dma_start(link_d[1][:], link_sb[:, 1, :])
            for l in range(3):
                nc.sync.dma_start(emb_d[l][:], emb_t[l][:])
                nc.sync.dma_start(cert_d[l][:], cert_t[l][:])

    nc.compile()
    return nc


def _get_nc():
    if "nc" not in _CACHE:
        _CACHE["nc"] = _build()
    return _CACHE["nc"]


def kernel(**inputs):
    from concourse.bass_utils import run_bass_kernel_spmd

    fmaps = [np.ascontiguousarray(np.asarray(inputs[f"fmap{l}"], dtype=np.float32))
             for l in range(3)]
    ws = [np.asarray(inputs[f"w{l}"], dtype=np.float32) for l in range(3)]
    bs = [np.asarray(inputs[f"b{l}"], dtype=np.float32) for l in range(3)]

    wTs = [np.ascontiguousarray(w.T) for w in ws]
    bias = np.ascontiguousarray(np.stack(bs, axis=1))  # (R, 3)

    nc = _get_nc()
    in_maps = []
    for i in range(NCORES):
        sl = slice(i * BLOC, (i + 1) * BLOC)
        m = {
            "bias": bias,
            "w0T": wTs[0], "w1T": wTs[1], "w2T": wTs[2],
        }
        for l in range(3):
            m[f"fmap{l}"] = np.ascontiguousarray(
                fmaps[l][sl].reshape(BLOC, CH[l], HW[l])
            )
        in_maps.append(m)

    trace = os.environ.get("BASS_KERNEL_TRACE") == "1"
    res = run_bass_kernel_spmd(nc, in_maps, core_ids=list(range(NCORES)),
                               trace=trace)
    _CACHE["last_result"] = res
    rs = res.results

    embs = [
        np.concatenate([rs[i][f"emb{l}"].T for i in range(NCORES)], axis=0)
        for l in range(3)
    ]
    certs = [
        np.concatenate([rs[i][f"cert{l}"].T for i in range(NCORES)], axis=0)
        for l in range(3)
    ]
    links = [
        (
            np.sum(
                np.stack([rs[i][f"link{l}"] for i in range(NCORES)]).astype(np.float64),
                axis=0,
            )
            / B
        ).astype(np.float32)
        for l in range(2)
    ]
    return (*embs, *certs, *links)


# revision 8
# speedup vs baseline: 32833.4493x; 32833.4493x over previous
"""AVSL-Graph fused kernel for Trainium2 (8 NeuronCores, data-parallel over batch).

Computation (per reference):
  for each level l in {0,1,2}:
    cam_l[b,r,hw] = sum_c w_l[r,c] * fmap_l[b,c,hw]          (1x1-conv GEMM)
    emb_l[b,r]    = mean_hw(cam_l) + bias_l[r]               (== pooled @ w.T + b)
    cert_l[b,r]   = std_hw(cam_l, ddof=1)
  link_l = sum_b  norm(pool2x2(cam_l))[b] @ norm(cam_{l+1})[b].T / B

Sharding: batch B=64 split 8 ways (8 samples/core). Each core computes its
embedding/certainty rows and an unnormalized link partial (sum over its local
samples, accumulated in PSUM); the host sums the 8 partials and divides by B.
"""

import os
import sys

import numpy as np

for _p in ("/opt/trn_rl_repo", "/root/.axon_site/_ro/trn_rl_repo"):
    if os.path.isdir(_p) and _p not in sys.path:
        sys.path.append(_p)

B = 64
R = 128
NCORES = 8
BLOC = B // NCORES  # 8 samples per core
CH = (512, 1024, 2048)
HW = (784, 196, 49)  # 28*28, 14*14, 7*7
KC = (4, 8, 16)  # contraction chunks of 128 per level
EPS = 1e-12

_CACHE = {}


def _build():
    import concourse.bacc as bacc
    import concourse.tile as tile
    from concourse import masks, mybir

    f32 = mybir.dt.float32
    AX = mybir.AxisListType
    AF = mybir.ActivationFunctionType

    nc = bacc.Bacc("TRN2", target_bir_lowering=False, debug=False)

    fm = [
        nc.dram_tensor(f"fmap{l}", (BLOC, CH[l], HW[l]), f32, kind="ExternalInput")
        for l in range(3)
    ]
    wT = [
        nc.dram_tensor(f"w{l}T", (CH[l], R), f32, kind="ExternalInput")
        for l in range(3)
    ]
    bias = nc.dram_tensor("bias", (R, 3), f32, kind="ExternalInput")
    emb_d = [
        nc.dram_tensor(f"emb{l}", (R, BLOC), f32, kind="ExternalOutput")
        for l in range(3)
    ]
    cert_d = [
        nc.dram_tensor(f"cert{l}", (R, BLOC), f32, kind="ExternalOutput")
        for l in range(3)
    ]
    link_d = [
        nc.dram_tensor(f"link{l}", (R, R), f32, kind="ExternalOutput")
        for l in range(2)
    ]

    with tile.TileContext(nc) as tc:
        with (
            tc.tile_pool(name="const", bufs=1) as const,
            tc.tile_pool(name="f0p", bufs=3) as f0p,
            tc.tile_pool(name="f1p", bufs=2) as f1p,
            tc.tile_pool(name="f2p", bufs=1) as f2p,
            tc.tile_pool(name="work", bufs=2) as work,
            tc.tile_pool(name="stats", bufs=2) as stats,
            tc.tile_pool(name="outp", bufs=1) as outp,
            tc.tile_pool(name="ps_cam0", bufs=1, space="PSUM") as ps_cam0,
            tc.tile_pool(name="ps_cam1", bufs=1, space="PSUM") as ps_cam1,
            tc.tile_pool(name="ps_cam2", bufs=1, space="PSUM") as ps_cam2,
            tc.tile_pool(name="ps_link", bufs=1, space="PSUM") as ps_link,
            tc.tile_pool(name="ps_tr", bufs=2, space="PSUM") as ps_tr,
        ):
            # ---- constants ----
            wt = []
            for l in range(3):
                t = const.tile([128, KC[l], 128], f32, tag=f"wt{l}", name=f"wt{l}")
                nc.sync.dma_start(t[:], wT[l][:].rearrange("(k p) r -> p k r", p=128))
                wt.append(t)
            bias_t = const.tile([128, 3], f32, tag="bias")
            nc.sync.dma_start(bias_t[:], bias[:])
            ident = const.tile([128, 128], f32, tag="ident")
            masks.make_identity(nc, ident[:])

            # ---- output accumulators in SBUF ----
            emb_t = [outp.tile([128, BLOC], f32, tag=f"emb{l}", name=f"emb_t{l}") for l in range(3)]
            cert_t = [outp.tile([128, BLOC], f32, tag=f"cert{l}", name=f"cert_t{l}") for l in range(3)]

            # ---- level 2 GEMM, all 8 samples at once (free dim = (b, hw) = 392) ----
            f2t = f2p.tile([128, KC[2], BLOC, HW[2]], f32, tag="f2")
            fm2v = fm[2][:].rearrange("b (k p) hw -> p k b hw", p=128)
            for k in range(KC[2]):
                nc.sync.dma_start(f2t[:, k, :, :], fm2v[:, k, :, :])
            cam2 = ps_cam2.tile([128, BLOC, HW[2]], f32, tag="cam2")
            for k in range(KC[2]):
                nc.tensor.matmul(
                    cam2[:],
                    wt[2][:, k, :],
                    f2t[:, k, :, :],
                    start=(k == 0),
                    stop=(k == KC[2] - 1),
                )

            link0_t = ps_link.tile([128, 128], f32, tag="link0")
            link1_t = ps_link.tile([128, 128], f32, tag="link1")

            cam1 = None
            for b in range(BLOC):
                # ---- level 1 GEMM per pair of samples (free dim 392) ----
                if b % 2 == 0:
                    f1t = f1p.tile([128, KC[1], 2, HW[1]], f32, tag="f1")
                    for j in range(2):
                        nc.sync.dma_start(
                            f1t[:, :, j, :],
                            fm[1][b + j].rearrange("(k p) hw -> p k hw", p=128),
                        )
                    cam1 = ps_cam1.tile([128, 2, HW[1]], f32, tag="cam1")
                    for k in range(KC[1]):
                        nc.tensor.matmul(
                            cam1[:],
                            wt[1][:, k, :],
                            f1t[:, k, :, :],
                            start=(k == 0),
                            stop=(k == KC[1] - 1),
                        )

                # ---- level 0 GEMM for this sample (2 halves of 392) ----
                f0t = f0p.tile([128, KC[0], HW[0]], f32, tag="f0")
                nc.sync.dma_start(
                    f0t[:], fm[0][b].rearrange("(k p) hw -> p k hw", p=128)
                )
                cam0 = ps_cam0.tile([128, 2, 512], f32, tag="cam0")
                for h in range(2):
                    for k in range(KC[0]):
                        nc.tensor.matmul(
                            cam0[:, h, 0:392],
                            wt[0][:, k, :],
                            f0t[:, k, h * 392 : (h + 1) * 392],
                            start=(k == 0),
                            stop=(k == KC[0] - 1),
                        )

                # ================= level 0 stats + pooled lo =================
                st0 = stats.tile([128, 2, 6], f32, tag="st0")
                for h in range(2):
                    nc.vector.bn_stats(st0[:, h, :], cam0[:, h, 0:392])
                mv0 = stats.tile([128, 2], f32, tag="mv0")
                nc.vector.bn_aggr(mv0[:], st0[:])
                nc.scalar.activation(
                    emb_t[0][:, b : b + 1], mv0[:, 0:1], AF.Identity,
                    bias=bias_t[:, 0:1], scale=1.0,
                )
                nc.scalar.activation(
                    cert_t[0][:, b : b + 1], mv0[:, 1:2], AF.Sqrt,
                    scale=float(HW[0]) / float(HW[0] - 1),
                )
                # 2x2 average pool (no 1/4 scale: it cancels in normalization)
                praw0 = work.tile([128, 196], f32, tag="praw0")
                for h in range(2):
                    nc.vector.reduce_sum(
                        praw0[:, h * 98 : (h + 1) * 98].rearrange(
                            "p (oh ow) -> p oh ow", oh=7
                        ),
                        cam0[:, h, 0:392].rearrange(
                            "p (oh dh ow dw) -> p oh ow dh dw", oh=7, dh=2, ow=14, dw=2
                        ),
                        axis=AX.XY,
                    )
                sq0 = work.tile([128, 196], f32, tag="sq0")
                q0 = stats.tile([128, 1], f32, tag="q0")
                nc.scalar.activation(sq0[:], praw0[:], AF.Square, accum_out=q0[:])
                inv_lo0 = stats.tile([128, 1], f32, tag="inv_lo0")
                nc.scalar.sqrt(inv_lo0[:], q0[:])
                nc.vector.tensor_scalar_max(inv_lo0[:], inv_lo0[:], EPS)
                nc.vector.reciprocal(inv_lo0[:], inv_lo0[:])
                lo0 = work.tile([128, 196], f32, tag="lo0")
                nc.vector.tensor_scalar_mul(lo0[:], praw0[:], inv_lo0[:])

                tr0 = ps_tr.tile([128, 4, 128], f32, tag="tr")
                nc.tensor.transpose(tr0[:, 0, :], lo0[:, 0:128], ident[:])
                nc.tensor.transpose(tr0[0:68, 1, :], lo0[:, 128:196], ident[:])

                # ================= level 1 stats + hi1 + pooled lo1 =================
                camb = cam1[:, b % 2, :]
                st1 = stats.tile([128, 6], f32, tag="st1")
                nc.vector.bn_stats(st1[:], camb)
                mv1 = stats.tile([128, 2], f32, tag="mv1")
                nc.vector.bn_aggr(mv1[:], st1[:])
                nc.scalar.activation(
                    emb_t[1][:, b : b + 1], mv1[:, 0:1], AF.Identity,
                    bias=bias_t[:, 1:2], scale=1.0,
                )
                nc.scalar.activation(
                    cert_t[1][:, b : b + 1], mv1[:, 1:2], AF.Sqrt,
                    scale=float(HW[1]) / float(HW[1] - 1),
                )
                # ||cam1||^2 = n*(var + mean^2)
                nh1 = stats.tile([128, 1], f32, tag="nh1")
                nc.vector.tensor_mul(nh1[:], mv1[:, 0:1], mv1[:, 0:1])
                nc.vector.tensor_add(nh1[:], nh1[:], mv1[:, 1:2])
                nc.scalar.activation(nh1[:], nh1[:], AF.Sqrt, scale=float(HW[1]))
                nc.vector.tensor_scalar_max(nh1[:], nh1[:], EPS)
                nc.vector.reciprocal(nh1[:], nh1[:])
                hi1 = work.tile([128, 196], f32, tag="hi1")
                nc.vector.tensor_scalar_mul(hi1[:], camb, nh1[:])
                nc.tensor.transpose(tr0[:, 2, :], hi1[:, 0:128], ident[:])
                nc.tensor.transpose(tr0[0:68, 3, :], hi1[:, 128:196], ident[:])

                praw1 = work.tile([128, 49], f32, tag="praw1")
                nc.vector.reduce_sum(
                    praw1[:].rearrange("p (oh ow) -> p oh ow", oh=7),
                    camb.rearrange(
                        "p (oh dh ow dw) -> p oh ow dh dw", oh=7, dh=2, ow=7, dw=2
                    ),
                    axis=AX.XY,
                )
                sq1 = work.tile([128, 49], f32, tag="sq1")
                q1 = stats.tile([128, 1], f32, tag="q1")
                nc.scalar.activation(sq1[:], praw1[:], AF.Square, accum_out=q1[:])
                inv_lo1 = stats.tile([128, 1], f32, tag="inv_lo1")
                nc.scalar.sqrt(inv_lo1[:], q1[:])
                nc.vector.tensor_scalar_max(inv_lo1[:], inv_lo1[:], EPS)
                nc.vector.reciprocal(inv_lo1[:], inv_lo1[:])
                lo1 = work.tile([128, 49], f32, tag="lo1")
                nc.vector.tensor_scalar_mul(lo1[:], praw1[:], inv_lo1[:])

                tr1 = ps_tr.tile([128, 4, 128], f32, tag="tr")
                nc.tensor.transpose(tr1[0:49, 0, :], lo1[:], ident[:])

                # ================= level 2 stats + hi2 =================
                cam2b = cam2[:, b, :]
                s2 = stats.tile([128, 1], f32, tag="s2")
                q2 = stats.tile([128, 1], f32, tag="q2")
                sc2 = work.tile([128, 49], f32, tag="sc2")
                nc.scalar.activation(sc2[:], cam2b, AF.Copy, accum_out=s2[:])
                sc2b = work.tile([128, 49], f32, tag="sc2b")
                nc.scalar.activation(sc2b[:], cam2b, AF.Square, accum_out=q2[:])
                nc.scalar.activation(
                    emb_t[2][:, b : b + 1], s2[:], AF.Identity,
                    bias=bias_t[:, 2:3], scale=1.0 / HW[2],
                )
                # cert2 = sqrt((q2 - s2^2/n) / (n-1))
                v2 = stats.tile([128, 1], f32, tag="v2")
                nc.vector.tensor_mul(v2[:], s2[:], s2[:])
                nc.vector.tensor_scalar_mul(v2[:], v2[:], -1.0 / HW[2])
                nc.vector.tensor_add(v2[:], v2[:], q2[:])
                nc.scalar.activation(
                    cert_t[2][:, b : b + 1], v2[:], AF.Sqrt,
                    scale=1.0 / (HW[2] - 1),
                )
                nh2 = stats.tile([128, 1], f32, tag="nh2")
                nc.scalar.sqrt(nh2[:], q2[:])
                nc.vector.tensor_scalar_max(nh2[:], nh2[:], EPS)
                nc.vector.reciprocal(nh2[:], nh2[:])
                hi2 = work.tile([128, 49], f32, tag="hi2")
                nc.vector.tensor_scalar_mul(hi2[:], cam2b, nh2[:])
                nc.tensor.transpose(tr1[0:49, 1, :], hi2[:], ident[:])

                # ---- PSUM -> SBUF copies of transposed operands ----
                loT0 = work.tile([128, 2, 128], f32, tag="loT0")
                nc.vector.tensor_copy(loT0[:, 0, :], tr0[:, 0, :])
                nc.vector.tensor_copy(loT0[0:68, 1, :], tr0[0:68, 1, :])
                hiT1 = work.tile([128, 2, 128], f32, tag="hiT1")
                nc.vector.tensor_copy(hiT1[:, 0, :], tr0[:, 2, :])
                nc.vector.tensor_copy(hiT1[0:68, 1, :], tr0[0:68, 3, :])
                loT1 = work.tile([128, 128], f32, tag="loT1")
                nc.vector.tensor_copy(loT1[0:49, :], tr1[0:49, 0, :])
                hiT2 = work.tile([128, 128], f32, tag="hiT2")
                nc.vector.tensor_copy(hiT2[0:49, :], tr1[0:49, 1, :])

                # ---- link partial accumulation over samples ----
                nc.tensor.matmul(
                    link0_t[:], loT0[:, 0, :], hiT1[:, 0, :],
                    start=(b == 0), stop=False,
                )
                nc.tensor.matmul(
                    link0_t[:], loT0[0:68, 1, :], hiT1[0:68, 1, :],
                    start=False, stop=(b == BLOC - 1),
                )
                nc.tensor.matmul(
                    link1_t[:], loT1[0:49, :], hiT2[0:49, :],
                    start=(b == 0), stop=(b == BLOC - 1),
                )

            # ---- write outputs ----
            link_sb = outp.tile([128, 2, 128], f32, tag="link_sb")
            nc.vector.tensor_copy(link_sb[:, 0, :], link0_t[:])
            nc.vector.tensor_copy(link_sb[:, 1, :], link1_t[:])
            nc.sync.dma_start(link_d[0][:], link_sb[:, 0, :])
            nc.sync.dma_start(link_d[1][:], link_sb[:, 1, :])
            for l in range(3):
                nc.sync.dma_start(emb_d[l][:], emb_t[l][:])
                nc.sync.dma_start(cert_d[l][:], cert_t[l][:])

    nc.compile()
    return nc


def _get_nc():
    if "nc" not in _CACHE:
        _CACHE["nc"] = _build()
    return _CACHE["nc"]


def kernel(**inputs):
    from concourse.bass_utils import run_bass_kernel_spmd

    fmaps = [np.ascontiguousarray(np.asarray(inputs[f"fmap{l}"], dtype=np.float32))
             for l in range(3)]
    ws = [np.asarray(inputs[f"w{l}"], dtype=np.float32) for l in range(3)]
    bs = [np.asarray(inputs[f"b{l}"], dtype=np.float32) for l in range(3)]

    wTs = [np.ascontiguousarray(w.T) for w in ws]
    bias = np.ascontiguousarray(np.stack(bs, axis=1))  # (R, 3)

    nc = _get_nc()
    in_maps = []
    for i in range(NCORES):
        sl = slice(i * BLOC, (i + 1) * BLOC)
        m = {
            "bias": bias,
            "w0T": wTs[0], "w1T": wTs[1], "w2T": wTs[2],
        }
        for l in range(3):
            m[f"fmap{l}"] = np.ascontiguousarray(
                fmaps[l][sl].reshape(BLOC, CH[l], HW[l])
            )
        in_maps.append(m)

    trace = os.environ.get("BASS_KERNEL_TRACE") == "1"
    kw = {}
    if trace and _CACHE.get("tmpdir"):
        kw["tmpdir"] = _CACHE["tmpdir"]
    res = run_bass_kernel_spmd(nc, in_maps, core_ids=list(range(NCORES)),
                               trace=trace, **kw)
    _CACHE["last_result"] = res
    rs = res.results

    embs = [
        np.concatenate([rs[i][f"emb{l}"].T for i in range(NCORES)], axis=0)
        for l in range(3)
    ]
    certs = [
        np.concatenate([rs[i][f"cert{l}"].T for i in range(NCORES)], axis=0)
        for l in range(3)
    ]
    links = [
        (
            np.sum(
                np.stack([rs[i][f"link{l}"] for i in range(NCORES)]).astype(np.float64),
                axis=0,
            )
            / B
        ).astype(np.float32)
        for l in range(2)
    ]
    return (*embs, *certs, *links)


# revision 9
# speedup vs baseline: 38234.8475x; 1.1645x over previous
"""AVSL-Graph fused kernel for Trainium2 (8 NeuronCores, data-parallel over batch).

Computation (per reference):
  for each level l in {0,1,2}:
    cam_l[b,r,hw] = sum_c w_l[r,c] * fmap_l[b,c,hw]          (1x1-conv GEMM)
    emb_l[b,r]    = mean_hw(cam_l) + bias_l[r]               (== pooled @ w.T + b)
    cert_l[b,r]   = std_hw(cam_l, ddof=1)
  link_l = sum_b  norm(pool2x2(cam_l))[b] @ norm(cam_{l+1})[b].T / B

Sharding: batch B=64 split 8 ways (8 samples/core). Each core computes its
embedding/certainty rows and an unnormalized link partial (sum over its local
samples, accumulated in PSUM); the host sums the 8 partials and divides by B.

The CAM GEMMs run with float32r operands (4-byte fp32 storage, reduced-precision
multiplier array at 4x the fp32 matmul rate; measured ~1.5e-4 rel err) with fp32
PSUM accumulation. Everything downstream (stats, norms, links) stays fp32.
"""

import os
import sys

import numpy as np

for _p in ("/opt/trn_rl_repo", "/root/.axon_site/_ro/trn_rl_repo"):
    if os.path.isdir(_p) and _p not in sys.path:
        sys.path.append(_p)

B = 64
R = 128
NCORES = 8
BLOC = B // NCORES  # 8 samples per core
CH = (512, 1024, 2048)
HW = (784, 196, 49)  # 28*28, 14*14, 7*7
KC = (4, 8, 16)  # contraction chunks of 128 per level

_CACHE = {}


def _build():
    import concourse.bacc as bacc
    import concourse.tile as tile
    from concourse import masks, mybir

    f32 = mybir.dt.float32
    f32r = mybir.dt.float32r
    AX = mybir.AxisListType
    AF = mybir.ActivationFunctionType

    nc = bacc.Bacc("TRN2", target_bir_lowering=False, debug=False)

    fm = [
        nc.dram_tensor(f"fmap{l}", (BLOC, CH[l], HW[l]), f32, kind="ExternalInput")
        for l in range(3)
    ]
    wT = [
        nc.dram_tensor(f"w{l}T", (CH[l], R), f32, kind="ExternalInput")
        for l in range(3)
    ]
    bias = nc.dram_tensor("bias", (R, 3), f32, kind="ExternalInput")
    emb_d = [
        nc.dram_tensor(f"emb{l}", (R, BLOC), f32, kind="ExternalOutput")
        for l in range(3)
    ]
    cert_d = [
        nc.dram_tensor(f"cert{l}", (R, BLOC), f32, kind="ExternalOutput")
        for l in range(3)
    ]
    link_d = [
        nc.dram_tensor(f"link{l}", (R, R), f32, kind="ExternalOutput")
        for l in range(2)
    ]

    with tile.TileContext(nc) as tc:
        with (
            tc.tile_pool(name="const", bufs=1) as const,
            tc.tile_pool(name="f0p", bufs=3) as f0p,
            tc.tile_pool(name="f1p", bufs=2) as f1p,
            tc.tile_pool(name="f2p", bufs=1) as f2p,
            tc.tile_pool(name="work", bufs=2) as work,
            tc.tile_pool(name="stats", bufs=2) as stats,
            tc.tile_pool(name="outp", bufs=1) as outp,
            tc.tile_pool(name="ps_cam0", bufs=1, space="PSUM") as ps_cam0,
            tc.tile_pool(name="ps_cam1", bufs=1, space="PSUM") as ps_cam1,
            tc.tile_pool(name="ps_cam2", bufs=1, space="PSUM") as ps_cam2,
            tc.tile_pool(name="ps_link", bufs=1, space="PSUM") as ps_link,
            tc.tile_pool(name="ps_tr", bufs=2, space="PSUM") as ps_tr,
        ):
            # ---- constants (weights cast to f32r during DMA for the GEMMs) ----
            wt = []
            for l in range(3):
                t = const.tile([128, KC[l], 128], f32r, tag=f"wt{l}", name=f"wt{l}")
                nc.gpsimd.dma_start(
                    t[:], wT[l][:].rearrange("(k p) r -> p k r", p=128)
                )
                wt.append(t)
            bias_t = const.tile([128, 3], f32, tag="bias")
            nc.sync.dma_start(bias_t[:], bias[:])
            ident = const.tile([128, 128], f32, tag="ident")
            masks.make_identity(nc, ident[:])

            # ---- output accumulators in SBUF ----
            emb_t = [outp.tile([128, BLOC], f32, tag=f"emb{l}", name=f"emb_t{l}")
                     for l in range(3)]
            cert_t = [outp.tile([128, BLOC], f32, tag=f"cert{l}", name=f"cert_t{l}")
                      for l in range(3)]
            # transposed normalized pooled cam1, one slot per sample (for link1)
            loT1s = outp.tile([128, BLOC, 128], f32, tag="loT1s")

            f2t = f2p.tile([128, KC[2], BLOC, HW[2]], f32r, tag="f2")
            fm2v = fm[2][:].rearrange("b (k p) hw -> p k b hw", p=128)

            link0_t = ps_link.tile([128, 128], f32, tag="link0")
            link1_t = ps_link.tile([128, 128], f32, tag="link1")
            cam2 = ps_cam2.tile([128, BLOC, HW[2]], f32, tag="cam2")

            # ============ phase A: levels 0+1, link0, save loT1 ============
            cam1 = None
            for b in range(BLOC):
                if b % 2 == 0:
                    f1t = f1p.tile([128, KC[1], 2, HW[1]], f32r, tag="f1")
                    for j in range(2):
                        nc.gpsimd.dma_start(
                            f1t[:, :, j, :],
                            fm[1][b + j].rearrange("(k p) hw -> p k hw", p=128),
                        )
                    cam1 = ps_cam1.tile([128, 2, HW[1]], f32, tag="cam1")
                    for k in range(KC[1]):
                        nc.tensor.matmul(
                            cam1[:],
                            wt[1][:, k, :],
                            f1t[:, k, :, :],
                            start=(k == 0),
                            stop=(k == KC[1] - 1),
                        )

                f0t = f0p.tile([128, KC[0], HW[0]], f32r, tag="f0")
                nc.gpsimd.dma_start(
                    f0t[:], fm[0][b].rearrange("(k p) hw -> p k hw", p=128)
                )
                # stream fmap2 in the background, 2 contraction chunks/iteration
                for k in (2 * b, 2 * b + 1):
                    nc.gpsimd.dma_start(f2t[:, k, :, :], fm2v[:, k, :, :])

                cam0 = ps_cam0.tile([128, 2, 512], f32, tag="cam0")
                for h in range(2):
                    for k in range(KC[0]):
                        nc.tensor.matmul(
                            cam0[:, h, 0:392],
                            wt[0][:, k, :],
                            f0t[:, k, h * 392 : (h + 1) * 392],
                            start=(k == 0),
                            stop=(k == KC[0] - 1),
                        )

                # ---- level 0 stats + pooled lo0 ----
                st0 = stats.tile([128, 2, 6], f32, tag="st0")
                for h in range(2):
                    nc.vector.bn_stats(st0[:, h, :], cam0[:, h, 0:392])
                mv0 = stats.tile([128, 2], f32, tag="mv0")
                nc.vector.bn_aggr(mv0[:], st0[:])
                nc.scalar.activation(
                    emb_t[0][:, b : b + 1], mv0[:, 0:1], AF.Identity,
                    bias=bias_t[:, 0:1], scale=1.0,
                )
                nc.scalar.activation(
                    cert_t[0][:, b : b + 1], mv0[:, 1:2], AF.Sqrt,
                    scale=float(HW[0]) / float(HW[0] - 1),
                )
                # 2x2 sum-pool (the 1/4 factor cancels in the normalization;
                # norms are O(1..100) >> the 1e-12 eps clamp, so it is omitted)
                praw0 = work.tile([128, 196], f32, tag="praw0")
                for h in range(2):
                    nc.vector.reduce_sum(
                        praw0[:, h * 98 : (h + 1) * 98].rearrange(
                            "p (oh ow) -> p oh ow", oh=7
                        ),
                        cam0[:, h, 0:392].rearrange(
                            "p (oh dh ow dw) -> p oh ow dh dw", oh=7, dh=2, ow=14, dw=2
                        ),
                        axis=AX.XY,
                    )
                sq0 = work.tile([128, 196], f32, tag="sq0")
                q0 = stats.tile([128, 1], f32, tag="q0")
                nc.scalar.activation(sq0[:], praw0[:], AF.Square, accum_out=q0[:])
                inv_lo0 = stats.tile([128, 1], f32, tag="inv_lo0")
                nc.scalar.sqrt(inv_lo0[:], q0[:])
                nc.vector.reciprocal(inv_lo0[:], inv_lo0[:])
                lo0 = work.tile([128, 196], f32, tag="lo0")
                nc.vector.tensor_scalar_mul(lo0[:], praw0[:], inv_lo0[:])

                tr0 = ps_tr.tile([128, 4, 128], f32, tag="tr")
                nc.tensor.transpose(tr0[:, 0, :], lo0[:, 0:128], ident[:])
                nc.tensor.transpose(tr0[0:68, 1, :], lo0[:, 128:196], ident[:])

                # ---- level 1 stats + hi1 + pooled lo1 ----
                camb = cam1[:, b % 2, :]
                st1 = stats.tile([128, 6], f32, tag="st1")
                nc.vector.bn_stats(st1[:], camb)
                mv1 = stats.tile([128, 2], f32, tag="mv1")
                nc.vector.bn_aggr(mv1[:], st1[:])
                nc.scalar.activation(
                    emb_t[1][:, b : b + 1], mv1[:, 0:1], AF.Identity,
                    bias=bias_t[:, 1:2], scale=1.0,
                )
                nc.scalar.activation(
                    cert_t[1][:, b : b + 1], mv1[:, 1:2], AF.Sqrt,
                    scale=float(HW[1]) / float(HW[1] - 1),
                )
                # ||cam1|| = sqrt(n*(var + mean^2))
                nh1 = stats.tile([128, 1], f32, tag="nh1")
                nc.gpsimd.tensor_mul(nh1[:], mv1[:, 0:1], mv1[:, 0:1])
                nc.gpsimd.tensor_add(nh1[:], nh1[:], mv1[:, 1:2])
                nc.scalar.activation(nh1[:], nh1[:], AF.Sqrt, scale=float(HW[1]))
                nc.vector.reciprocal(nh1[:], nh1[:])
                hi1 = work.tile([128, 196], f32, tag="hi1")
                nc.vector.tensor_scalar_mul(hi1[:], camb, nh1[:])
                nc.tensor.transpose(tr0[:, 2, :], hi1[:, 0:128], ident[:])
                nc.tensor.transpose(tr0[0:68, 3, :], hi1[:, 128:196], ident[:])

                praw1 = work.tile([128, 49], f32, tag="praw1")
                nc.vector.reduce_sum(
                    praw1[:].rearrange("p (oh ow) -> p oh ow", oh=7),
                    camb.rearrange(
                        "p (oh dh ow dw) -> p oh ow dh dw", oh=7, dh=2, ow=7, dw=2
                    ),
                    axis=AX.XY,
                )
                sq1 = work.tile([128, 49], f32, tag="sq1")
                q1 = stats.tile([128, 1], f32, tag="q1")
                nc.scalar.activation(sq1[:], praw1[:], AF.Square, accum_out=q1[:])
                inv_lo1 = stats.tile([128, 1], f32, tag="inv_lo1")
                nc.scalar.sqrt(inv_lo1[:], q1[:])
                nc.vector.reciprocal(inv_lo1[:], inv_lo1[:])
                lo1 = work.tile([128, 49], f32, tag="lo1")
                nc.vector.tensor_scalar_mul(lo1[:], praw1[:], inv_lo1[:])

                tr1 = ps_tr.tile([128, 4, 128], f32, tag="tr")
                nc.tensor.transpose(tr1[0:49, 0, :], lo1[:], ident[:])

                # ---- PSUM -> SBUF: one merged copy for the 4 tr0 slots ----
                loHiT = work.tile([128, 4, 128], f32, tag="loHiT")
                nc.vector.tensor_copy(loHiT[:], tr0[:])
                nc.vector.tensor_copy(loT1s[:, b, :], tr1[:, 0, :])

                # ---- link0 partial accumulation ----
                nc.tensor.matmul(
                    link0_t[:], loHiT[:, 0, :], loHiT[:, 2, :],
                    start=(b == 0), stop=False,
                )
                nc.tensor.matmul(
                    link0_t[:], loHiT[0:68, 1, :], loHiT[0:68, 3, :],
                    start=False, stop=(b == BLOC - 1),
                )

            # ============ phase B: level 2 + link1 ============
            for k in range(KC[2]):
                nc.tensor.matmul(
                    cam2[:],
                    wt[2][:, k, :],
                    f2t[:, k, :, :],
                    start=(k == 0),
                    stop=(k == KC[2] - 1),
                )

            for b in range(BLOC):
                cam2b = cam2[:, b, :]
                s2 = stats.tile([128, 1], f32, tag="s2")
                q2 = stats.tile([128, 1], f32, tag="q2")
                sc2 = work.tile([128, 49], f32, tag="sc2")
                nc.scalar.activation(sc2[:], cam2b, AF.Copy, accum_out=s2[:])
                sc2b = work.tile([128, 49], f32, tag="sc2b")
                nc.scalar.activation(sc2b[:], cam2b, AF.Square, accum_out=q2[:])
                nc.scalar.activation(
                    emb_t[2][:, b : b + 1], s2[:], AF.Identity,
                    bias=bias_t[:, 2:3], scale=1.0 / HW[2],
                )
                # cert2 = sqrt((q2 - s2^2/n) / (n-1))
                v2 = stats.tile([128, 1], f32, tag="v2")
                nc.gpsimd.tensor_mul(v2[:], s2[:], s2[:])
                nc.gpsimd.tensor_scalar_mul(v2[:], v2[:], -1.0 / HW[2])
                nc.gpsimd.tensor_add(v2[:], v2[:], q2[:])
                nc.scalar.activation(
                    cert_t[2][:, b : b + 1], v2[:], AF.Sqrt,
                    scale=1.0 / (HW[2] - 1),
                )
                nh2 = stats.tile([128, 1], f32, tag="nh2")
                nc.scalar.sqrt(nh2[:], q2[:])
                nc.vector.reciprocal(nh2[:], nh2[:])
                hi2 = work.tile([128, 49], f32, tag="hi2")
                nc.vector.tensor_scalar_mul(hi2[:], cam2b, nh2[:])

                tr2 = ps_tr.tile([128, 4, 128], f32, tag="tr")
                nc.tensor.transpose(tr2[0:49, 0, :], hi2[:], ident[:])
                hiT2 = work.tile([128, 128], f32, tag="hiT2")
                nc.vector.tensor_copy(hiT2[0:49, :], tr2[0:49, 0, :])

                nc.tensor.matmul(
                    link1_t[:], loT1s[0:49, b, :], hiT2[0:49, :],
                    start=(b == 0), stop=(b == BLOC - 1),
                )

            # ---- write outputs ----
            link_sb = outp.tile([128, 2, 128], f32, tag="link_sb")
            nc.vector.tensor_copy(link_sb[:, 0, :], link0_t[:])
            nc.vector.tensor_copy(link_sb[:, 1, :], link1_t[:])
            nc.sync.dma_start(link_d[0][:], link_sb[:, 0, :])
            nc.sync.dma_start(link_d[1][:], link_sb[:, 1, :])
            for l in range(3):
                nc.sync.dma_start(emb_d[l][:], emb_t[l][:])
                nc.sync.dma_start(cert_d[l][:], cert_t[l][:])

    nc.compile()
    return nc


def _get_nc():
    if "nc" not in _CACHE:
        _CACHE["nc"] = _build()
    return _CACHE["nc"]


def kernel(**inputs):
    from concourse.bass_utils import run_bass_kernel_spmd

    fmaps = [np.ascontiguousarray(np.asarray(inputs[f"fmap{l}"], dtype=np.float32))
             for l in range(3)]
    ws = [np.asarray(inputs[f"w{l}"], dtype=np.float32) for l in range(3)]
    bs = [np.asarray(inputs[f"b{l}"], dtype=np.float32) for l in range(3)]

    wTs = [np.ascontiguousarray(w.T) for w in ws]
    bias = np.ascontiguousarray(np.stack(bs, axis=1))  # (R, 3)

    nc = _get_nc()
    in_maps = []
    for i in range(NCORES):
        sl = slice(i * BLOC, (i + 1) * BLOC)
        m = {
            "bias": bias,
            "w0T": wTs[0], "w1T": wTs[1], "w2T": wTs[2],
        }
        for l in range(3):
            m[f"fmap{l}"] = np.ascontiguousarray(
                fmaps[l][sl].reshape(BLOC, CH[l], HW[l])
            )
        in_maps.append(m)

    trace = os.environ.get("BASS_KERNEL_TRACE") == "1"
    kw = {}
    if trace and _CACHE.get("tmpdir"):
        kw["tmpdir"] = _CACHE["tmpdir"]
    res = run_bass_kernel_spmd(nc, in_maps, core_ids=list(range(NCORES)),
                               trace=trace, **kw)
    _CACHE["last_result"] = res
    rs = res.results

    embs = [
        np.concatenate([rs[i][f"emb{l}"].T for i in range(NCORES)], axis=0)
        for l in range(3)
    ]
    certs = [
        np.concatenate([rs[i][f"cert{l}"].T for i in range(NCORES)], axis=0)
        for l in range(3)
    ]
    links = [
        (
            np.sum(
                np.stack([rs[i][f"link{l}"] for i in range(NCORES)]).astype(np.float64),
                axis=0,
            )
            / B
        ).astype(np.float32)
        for l in range(2)
    ]
    return (*embs, *certs, *links)


# revision 10
# speedup vs baseline: 44625.4561x; 1.1671x over previous
"""AVSL-Graph fused kernel for Trainium2 (8 NeuronCores, data-parallel over batch).

Computation (per reference):
  for each level l in {0,1,2}:
    cam_l[b,r,hw] = sum_c w_l[r,c] * fmap_l[b,c,hw]          (1x1-conv GEMM)
    emb_l[b,r]    = mean_hw(cam_l) + bias_l[r]               (== pooled @ w.T + b)
    cert_l[b,r]   = std_hw(cam_l, ddof=1)
  link_l = sum_b  norm(pool2x2(cam_l))[b] @ norm(cam_{l+1})[b].T / B

Sharding: batch B=64 split 8 ways (8 samples/core). Each core computes its
embedding/certainty rows and an unnormalized link partial (sum over its local
samples, accumulated in PSUM); the host sums the 8 partials and divides by B.

The CAM GEMMs run with float32r operands (4-byte fp32 storage, reduced-precision
multiplier array at 4x the fp32 matmul rate; measured ~1.5e-4 rel err) with fp32
PSUM accumulation. Everything downstream (stats, norms, links) stays fp32.
"""

import os
import sys

import numpy as np

for _p in ("/opt/trn_rl_repo", "/root/.axon_site/_ro/trn_rl_repo"):
    if os.path.isdir(_p) and _p not in sys.path:
        sys.path.append(_p)

B = 64
R = 128
NCORES = 8
BLOC = B // NCORES  # 8 samples per core
CH = (512, 1024, 2048)
HW = (784, 196, 49)  # 28*28, 14*14, 7*7
KC = (4, 8, 16)  # contraction chunks of 128 per level

_CACHE = {}


def _build():
    import concourse.bass as bass
    import concourse.bacc as bacc
    import concourse.tile as tile
    from concourse import masks, mybir

    f32 = mybir.dt.float32
    f32r = mybir.dt.float32r
    AX = mybir.AxisListType
    AF = mybir.ActivationFunctionType

    nc = bacc.Bacc("TRN2", target_bir_lowering=False, debug=False)

    fm = [
        nc.dram_tensor(f"fmap{l}", (BLOC, CH[l], HW[l]), f32r, kind="ExternalInput")
        for l in range(3)
    ]
    wT = [
        nc.dram_tensor(f"w{l}T", (CH[l], R), f32r, kind="ExternalInput")
        for l in range(3)
    ]
    bias = nc.dram_tensor("bias", (R, 3), f32, kind="ExternalInput")
    emb_d = [
        nc.dram_tensor(f"emb{l}", (R, BLOC), f32, kind="ExternalOutput")
        for l in range(3)
    ]
    cert_d = [
        nc.dram_tensor(f"cert{l}", (R, BLOC), f32, kind="ExternalOutput")
        for l in range(3)
    ]
    link_d = [
        nc.dram_tensor(f"link{l}", (R, R), f32, kind="ExternalOutput")
        for l in range(2)
    ]

    with tile.TileContext(nc) as tc:
        with (
            tc.tile_pool(name="const", bufs=1) as const,
            tc.tile_pool(name="f0p", bufs=3) as f0p,
            tc.tile_pool(name="f1p", bufs=2) as f1p,
            tc.tile_pool(name="f2p", bufs=1) as f2p,
            tc.tile_pool(name="work", bufs=2) as work,
            tc.tile_pool(name="stats", bufs=2) as stats,
            tc.tile_pool(name="outp", bufs=1) as outp,
            tc.tile_pool(name="ps_cam0", bufs=1, space="PSUM") as ps_cam0,
            tc.tile_pool(name="ps_cam1", bufs=1, space="PSUM") as ps_cam1,
            tc.tile_pool(name="ps_cam2", bufs=1, space="PSUM") as ps_cam2,
            tc.tile_pool(name="ps_link", bufs=1, space="PSUM") as ps_link,
            tc.tile_pool(name="ps_tr", bufs=2, space="PSUM") as ps_tr,
        ):
            # ---- constants (weights cast to f32r during DMA for the GEMMs) ----
            wt = []
            for l in range(3):
                t = const.tile([128, KC[l], 128], f32r, tag=f"wt{l}", name=f"wt{l}")
                nc.sync.dma_start(
                    t[:], wT[l][:].rearrange("(k p) r -> p k r", p=128)
                )
                wt.append(t)
            bias_t = const.tile([128, 3], f32, tag="bias")
            nc.sync.dma_start(bias_t[:], bias[:])
            ident = const.tile([128, 128], f32, tag="ident")
            masks.make_identity(nc, ident[:])

            # ---- output accumulators in SBUF ----
            emb_t = [outp.tile([128, BLOC], f32, tag=f"emb{l}", name=f"emb_t{l}")
                     for l in range(3)]
            cert_t = [outp.tile([128, BLOC], f32, tag=f"cert{l}", name=f"cert_t{l}")
                      for l in range(3)]
            # transposed normalized pooled cam1, one slot per sample (for link1)
            loT1s = outp.tile([128, BLOC, 128], f32, tag="loT1s")

            f2t = f2p.tile([128, KC[2], BLOC, HW[2]], f32r, tag="f2")
            fm2v = fm[2][:].rearrange("b (k p) hw -> p k b hw", p=128)

            link0_t = ps_link.tile([128, 128], f32, tag="link0")
            link1_t = ps_link.tile([128, 128], f32, tag="link1")
            cam2 = ps_cam2.tile([128, BLOC, HW[2]], f32, tag="cam2")

            # ============ phase A: levels 0+1, link0, save loT1 ============
            cam1 = None
            for b in range(BLOC):
                if b % 2 == 0:
                    f1t = f1p.tile([128, KC[1], 2, HW[1]], f32r, tag="f1")
                    for j in range(2):
                        nc.sync.dma_start(
                            f1t[:, :, j, :],
                            fm[1][b + j].rearrange("(k p) hw -> p k hw", p=128),
                        )
                    cam1 = ps_cam1.tile([128, 2, HW[1]], f32, tag="cam1")
                    for k in range(KC[1]):
                        nc.tensor.matmul(
                            cam1[:],
                            wt[1][:, k, :],
                            f1t[:, k, :, :],
                            start=(k == 0),
                            stop=(k == KC[1] - 1),
                        )

                f0t = f0p.tile([128, KC[0], HW[0]], f32r, tag="f0")
                nc.sync.dma_start(
                    f0t[:], fm[0][b].rearrange("(k p) hw -> p k hw", p=128)
                )
                # stream fmap2 in the background, 2 contraction chunks/iteration
                for k in (2 * b, 2 * b + 1):
                    nc.sync.dma_start(f2t[:, k, :, :], fm2v[:, k, :, :])

                cam0 = ps_cam0.tile([128, 2, 512], f32, tag="cam0")
                for h in range(2):
                    for k in range(KC[0]):
                        nc.tensor.matmul(
                            cam0[:, h, 0:392],
                            wt[0][:, k, :],
                            f0t[:, k, h * 392 : (h + 1) * 392],
                            start=(k == 0),
                            stop=(k == KC[0] - 1),
                        )

                # ---- level 0 stats + pooled lo0 ----
                st0 = stats.tile([128, 2, 6], f32, tag="st0")
                for h in range(2):
                    nc.vector.bn_stats(st0[:, h, :], cam0[:, h, 0:392])
                mv0 = stats.tile([128, 2], f32, tag="mv0")
                nc.vector.bn_aggr(mv0[:], st0[:])
                nc.scalar.activation(
                    emb_t[0][:, b : b + 1], mv0[:, 0:1], AF.Identity,
                    bias=bias_t[:, 0:1], scale=1.0,
                )
                nc.scalar.activation(
                    cert_t[0][:, b : b + 1], mv0[:, 1:2], AF.Sqrt,
                    scale=float(HW[0]) / float(HW[0] - 1),
                )
                # 2x2 sum-pool (the 1/4 factor cancels in the normalization;
                # norms are O(1..100) >> the 1e-12 eps clamp, so it is omitted)
                praw0 = work.tile([128, 196], f32, tag="praw0")
                for h in range(2):
                    nc.vector.reduce_sum(
                        praw0[:, h * 98 : (h + 1) * 98].rearrange(
                            "p (oh ow) -> p oh ow", oh=7
                        ),
                        cam0[:, h, 0:392].rearrange(
                            "p (oh dh ow dw) -> p oh ow dh dw", oh=7, dh=2, ow=14, dw=2
                        ),
                        axis=AX.XY,
                    )
                sq0 = work.tile([128, 196], f32, tag="sq0")
                q0 = stats.tile([128, 1], f32, tag="q0")
                nc.scalar.activation(sq0[:], praw0[:], AF.Square, accum_out=q0[:])
                inv_lo0 = stats.tile([128, 1], f32, tag="inv_lo0")
                nc.scalar.sqrt(inv_lo0[:], q0[:])
                nc.vector.reciprocal(inv_lo0[:], inv_lo0[:])
                lo0 = work.tile([128, 196], f32, tag="lo0")
                nc.vector.tensor_scalar_mul(lo0[:], praw0[:], inv_lo0[:])

                tr0 = ps_tr.tile([128, 4, 128], f32, tag="tr")
                nc.tensor.transpose(tr0[:, 0, :], lo0[:, 0:128], ident[:])
                nc.tensor.transpose(tr0[0:68, 1, :], lo0[:, 128:196], ident[:])

                # ---- level 1 stats + hi1 + pooled lo1 ----
                camb = cam1[:, b % 2, :]
                st1 = stats.tile([128, 6], f32, tag="st1")
                nc.vector.bn_stats(st1[:], camb)
                mv1 = stats.tile([128, 2], f32, tag="mv1")
                nc.vector.bn_aggr(mv1[:], st1[:])
                nc.scalar.activation(
                    emb_t[1][:, b : b + 1], mv1[:, 0:1], AF.Identity,
                    bias=bias_t[:, 1:2], scale=1.0,
                )
                nc.scalar.activation(
                    cert_t[1][:, b : b + 1], mv1[:, 1:2], AF.Sqrt,
                    scale=float(HW[1]) / float(HW[1] - 1),
                )
                # ||cam1|| = sqrt(n*(var + mean^2))
                nh1 = stats.tile([128, 1], f32, tag="nh1")
                nc.gpsimd.tensor_mul(nh1[:], mv1[:, 0:1], mv1[:, 0:1])
                nc.gpsimd.tensor_add(nh1[:], nh1[:], mv1[:, 1:2])
                nc.scalar.activation(nh1[:], nh1[:], AF.Sqrt, scale=float(HW[1]))
                nc.vector.reciprocal(nh1[:], nh1[:])
                hi1 = work.tile([128, 196], f32, tag="hi1")
                nc.vector.tensor_scalar_mul(hi1[:], camb, nh1[:])
                nc.tensor.transpose(tr0[:, 2, :], hi1[:, 0:128], ident[:])
                nc.tensor.transpose(tr0[0:68, 3, :], hi1[:, 128:196], ident[:])

                praw1 = work.tile([128, 49], f32, tag="praw1")
                nc.vector.reduce_sum(
                    praw1[:].rearrange("p (oh ow) -> p oh ow", oh=7),
                    camb.rearrange(
                        "p (oh dh ow dw) -> p oh ow dh dw", oh=7, dh=2, ow=7, dw=2
                    ),
                    axis=AX.XY,
                )
                sq1 = work.tile([128, 49], f32, tag="sq1")
                q1 = stats.tile([128, 1], f32, tag="q1")
                nc.scalar.activation(sq1[:], praw1[:], AF.Square, accum_out=q1[:])
                inv_lo1 = stats.tile([128, 1], f32, tag="inv_lo1")
                nc.scalar.sqrt(inv_lo1[:], q1[:])
                nc.vector.reciprocal(inv_lo1[:], inv_lo1[:])
                lo1 = work.tile([128, 49], f32, tag="lo1")
                nc.vector.tensor_scalar_mul(lo1[:], praw1[:], inv_lo1[:])

                tr1 = ps_tr.tile([128, 4, 128], f32, tag="tr")
                nc.tensor.transpose(tr1[0:49, 0, :], lo1[:], ident[:])

                # ---- PSUM -> SBUF: one merged copy for the 4 tr0 slots ----
                loHiT = work.tile([128, 4, 128], f32, tag="loHiT")
                nc.vector.tensor_copy(loHiT[:], tr0[:])
                nc.vector.tensor_copy(loT1s[:, b, :], tr1[:, 0, :])

                # ---- link0 partial accumulation ----
                nc.tensor.matmul(
                    link0_t[:], loHiT[:, 0, :], loHiT[:, 2, :],
                    start=(b == 0), stop=False,
                )
                nc.tensor.matmul(
                    link0_t[:], loHiT[0:68, 1, :], loHiT[0:68, 3, :],
                    start=False, stop=(b == BLOC - 1),
                )

            # ============ phase B: level 2 + link1 ============
            for k in range(KC[2]):
                nc.tensor.matmul(
                    cam2[:],
                    wt[2][:, k, :],
                    f2t[:, k, :, :],
                    start=(k == 0),
                    stop=(k == KC[2] - 1),
                )

            sqa = work.tile([128, BLOC, HW[2]], f32, tag="sqa")
            nc.scalar.activation(sqa[:], cam2[:], AF.Square)
            S2 = stats.tile([128, BLOC], f32, tag="S2")
            nc.vector.reduce_sum(S2[:], cam2[:], axis=AX.X)
            Q2 = stats.tile([128, BLOC], f32, tag="Q2")
            nc.vector.reduce_sum(Q2[:], sqa[:], axis=AX.X)
            nc.vector.tensor_scalar(
                emb_t[2][:], S2[:], 1.0 / HW[2], bias_t[:, 2:3],
                op0=mybir.AluOpType.mult, op1=mybir.AluOpType.add,
            )
            v2a = stats.tile([128, BLOC], f32, tag="v2a")
            nc.gpsimd.tensor_mul(v2a[:], S2[:], S2[:])
            nc.gpsimd.tensor_scalar_mul(v2a[:], v2a[:], -1.0 / HW[2])
            nc.gpsimd.tensor_add(v2a[:], v2a[:], Q2[:])
            nc.scalar.activation(
                cert_t[2][:], v2a[:], AF.Sqrt, scale=1.0 / (HW[2] - 1)
            )
            nh2a = stats.tile([128, BLOC], f32, tag="nh2a")
            nc.scalar.sqrt(nh2a[:], Q2[:])
            nc.vector.reciprocal(nh2a[:], nh2a[:])
            hi2a = work.tile([128, BLOC, HW[2]], f32, tag="hi2a")
            nh2v = nh2a[:]
            nh2b = bass.AP(
                tensor=nh2v.tensor, offset=nh2v.offset,
                ap=[nh2v.ap[0], nh2v.ap[1], [0, HW[2]]],
            )
            nc.vector.tensor_mul(hi2a[:], cam2[:], nh2b)

            hiT2s = []
            for g in range(2):
                trg = ps_tr.tile([128, 4, 128], f32, tag="tr", name=f"tr2_{g}")
                for j in range(4):
                    nc.tensor.transpose(
                        trg[0:49, j, :], hi2a[:, 4 * g + j, :], ident[:]
                    )
                ht = work.tile([128, 4, 128], f32, tag="hiT2s", name=f"hiT2s{g}")
                nc.vector.tensor_copy(ht[:], trg[:])
                hiT2s.append(ht)
            for b in range(BLOC):
                nc.tensor.matmul(
                    link1_t[:], loT1s[0:49, b, :], hiT2s[b // 4][0:49, b % 4, :],
                    start=(b == 0), stop=(b == BLOC - 1),
                )

            # ---- write outputs ----
            link_sb = outp.tile([128, 2, 128], f32, tag="link_sb")
            nc.vector.tensor_copy(link_sb[:, 0, :], link0_t[:])
            nc.vector.tensor_copy(link_sb[:, 1, :], link1_t[:])
            nc.sync.dma_start(link_d[0][:], link_sb[:, 0, :])
            nc.sync.dma_start(link_d[1][:], link_sb[:, 1, :])
            for l in range(3):
                nc.sync.dma_start(emb_d[l][:], emb_t[l][:])
                nc.sync.dma_start(cert_d[l][:], cert_t[l][:])

    nc.compile()
    return nc


def _get_nc():
    if "nc" not in _CACHE:
        _CACHE["nc"] = _build()
    return _CACHE["nc"]


def kernel(**inputs):
    from concourse.bass_utils import run_bass_kernel_spmd

    fmaps = [np.ascontiguousarray(np.asarray(inputs[f"fmap{l}"], dtype=np.float32))
             for l in range(3)]
    ws = [np.asarray(inputs[f"w{l}"], dtype=np.float32) for l in range(3)]
    bs = [np.asarray(inputs[f"b{l}"], dtype=np.float32) for l in range(3)]

    wTs = [np.ascontiguousarray(w.T) for w in ws]
    bias = np.ascontiguousarray(np.stack(bs, axis=1))  # (R, 3)

    nc = _get_nc()
    in_maps = []
    for i in range(NCORES):
        sl = slice(i * BLOC, (i + 1) * BLOC)
        m = {
            "bias": bias,
            "w0T": wTs[0], "w1T": wTs[1], "w2T": wTs[2],
        }
        for l in range(3):
            m[f"fmap{l}"] = np.ascontiguousarray(
                fmaps[l][sl].reshape(BLOC, CH[l], HW[l])
            )
        in_maps.append(m)

    trace = os.environ.get("BASS_KERNEL_TRACE") == "1"
    kw = {}
    if trace and _CACHE.get("tmpdir"):
        kw["tmpdir"] = _CACHE["tmpdir"]
    res = run_bass_kernel_spmd(nc, in_maps, core_ids=list(range(NCORES)),
                               trace=trace, **kw)
    _CACHE["last_result"] = res
    rs = res.results

    embs = [
        np.concatenate([rs[i][f"emb{l}"].T for i in range(NCORES)], axis=0)
        for l in range(3)
    ]
    certs = [
        np.concatenate([rs[i][f"cert{l}"].T for i in range(NCORES)], axis=0)
        for l in range(3)
    ]
    links = [
        (
            np.sum(
                np.stack([rs[i][f"link{l}"] for i in range(NCORES)]).astype(np.float64),
                axis=0,
            )
            / B
        ).astype(np.float32)
        for l in range(2)
    ]
    return (*embs, *certs, *links)
